# revision 69
# baseline (speedup 1.0000x reference)
"""Multi-head attention (B=2, S=2048, D=1024, H=16) on 8 TRN2 NeuronCores.

Sharding: 2-way data parallel over batch x 4-way tensor parallel over heads
(4 heads = 256 dims per core).  Each core computes, for its (batch, head
group): Q/K/V projections, causal attention, and a partial output
projection (row-sharded Wo).  The host sums the 4 partials per batch and
adds bo.

Pipelined structure: the causal mask means attention chunk i only needs
projections for seq chunks <= i, so projections for chunk i+1 (PE-heavy,
ACT-idle) are interleaved as "filler" into attention chunk i (ACT-bound:
the exp activations are the per-chunk critical path).  Likewise the
norm+Wo work for chunk i-1 fills chunk i.  Inputs are packed per seq
chunk on the host so the DMA stream delivers exactly what the next
projection needs; compute starts after ~1.5MB instead of ~15MB.

Device layout notes:
  - All projections produce "head-transposed" activations qh^T/kh^T
    [head_dim, S] so the scores matmul scoresT[t, s] = kh @ qh^T needs no
    on-chip transposes.  V is produced in natural layout [S, head_dim] with
    an appended ones column, so the AV matmul also computes the softmax
    denominator (row 64 of its PSUM output) for free.
  - The two heads of a pair occupy partitions 0-63 / 64-127, so their
    K=64 scores matmuls sit in different PE row groups and execute
    concurrently (row-tiled array).
  - Scores are bounded (~N(0,1)), so softmax needs no max subtraction:
    attn = exp(s/8) * mask, normalized by the matmul-computed denominator.
  - The mask is handled on the host: each [128 key, 512 query] scoresT
    block gets an active column range [lo, hi) (fully-masked columns are
    never computed) plus optional 128-column multiplicative bf16 mask
    tiles.  Works for any mask; for the causal mask this degenerates to
    one shared triangular tile and ~38% less score work.
  - Softmax normalization: a fast approximate reciprocal runs on each
    aoU's denominator row in place (no cross-partition DMA gather), the
    bf16 results live in a persistent 2-chunk-rotating store, and K=1
    bf16 matmuls broadcast each row across 64 partitions.  Output
    partials are written f16 (host sums in f32).
  - AV matmuls are emitted two (first chunks: four) j-iterations behind
    their scores matmuls so the in-order PE never drains on the ACT
    engine's exp or on late-arriving v tensors.
  - Input DMA rides the sync+gpsimd rings only: a ring arms its next
    transfer when the previous completes, so the arming engine is blocked
    for the whole input stream — the scalar (ACT) engine must stay free
    or every exp queues behind the armings.  Each chunk's q/k (which gate
    the serial exp chain) are ordered before its v.
  - The exp ACT table is preloaded at kernel start, and dummy matmuls
    keep the PE's DVFS ramp warm while the first input slices arrive
    (the PE runs at 0.65-1.2GHz until ~8us into a continuous streak).
"""

import sys

sys.path.insert(0, "/opt/trn_rl_repo")

from contextlib import ExitStack

import ml_dtypes
import numpy as np

B, S, D, H = 2, 2048, 1024, 16
DK = D // H            # 64
NCORE = 8
DPB = 2                # data-parallel ways (batch)
TPG = NCORE // DPB     # 4 head groups
GH = H // TPG          # 4 heads per group
GD = GH * DK           # 256 dims per group
NPAIR = GH // 2        # 2 head pairs per group
SQC = 512              # Sq chunk (matmul moving dim)
SKC = 128              # Skv chunk (matmul partition dim)
MCH = 128              # mask chunk width
NI = S // SQC          # 4
NJ = S // SKC          # 16
KCH = D // 128         # 8 contraction chunks for the projections

TRACE = False
LAST_EXEC_NS = None
LAST_RESULT = None

_BF = ml_dtypes.bfloat16
_prog_cache = {}


def _classify_mask(mask_st):
    """mask_st: [S, S] bool indexed [query s, key t].

    Returns (cls, tiles): cls[i][j] is None (skip) or a dict with
      lo, hi : active scoresT column range (multiples of MCH)
      muls   : list of (col_off, tile_idx) 128-col multiplicative masks
    tiles: deduped bf16 [SKC, MCH] tiles in scoresT orientation [t, s].
    """
    cls = [[None] * NJ for _ in range(NI)]
    tiles = []
    keys = {}

    def tile_idx(sub):
        t = np.ascontiguousarray(sub.T)  # [SKC t, MCH s]
        key = t.tobytes()
        if key not in keys:
            keys[key] = len(tiles)
            tiles.append(t.astype(_BF))
        return keys[key]

    for i in range(NI):
        sblk = mask_st[i * SQC : (i + 1) * SQC]
        for j in range(NJ):
            blk = sblk[:, j * SKC : (j + 1) * SKC]  # [SQC s, SKC t]
            any_col = blk.any(axis=1)               # per query col of scoresT
            if not any_col.any():
                continue
            nz = np.nonzero(any_col)[0]
            lo = (int(nz[0]) // MCH) * MCH
            hi = -(-(int(nz[-1]) + 1) // MCH) * MCH
            muls = []
            for c in range(lo, hi, MCH):
                sub = blk[c : c + MCH]              # [MCH s, SKC t]
                if not sub.all():
                    muls.append((c, tile_idx(sub)))
            cls[i][j] = {"lo": lo, "hi": hi, "muls": muls}
    return cls, tiles


def _build(cls, n_mask, with_bias):
    """Build the (SPMD, per-core) Bass program."""
    import concourse.bacc as bacc
    import concourse.tile as tile
    from concourse import mybir

    BF = mybir.dt.bfloat16
    F16 = mybir.dt.float16
    F32 = mybir.dt.float32
    AF = mybir.ActivationFunctionType

    nc = bacc.Bacc("TRN2", target_bir_lowering=False, debug=False)

    # x inputs packed per seq chunk: row block sc holds, for each kk, the
    # 128 contraction rows of x^T restricted to that chunk's 512 columns
    xqT = nc.dram_tensor("xqT", [NI * 128, KCH * SQC], BF, kind="ExternalInput").ap()
    xkT = nc.dram_tensor("xkT", [NI * 128, KCH * SQC], BF, kind="ExternalInput").ap()
    xvT = nc.dram_tensor("xvT", [NI * 128, KCH * SQC], BF, kind="ExternalInput").ap()
    # packed weights: [128, KCH*GD], chunk kk at cols [kk*GD, (kk+1)*GD)
    wq_d = nc.dram_tensor("WQ", [128, KCH * GD], BF, kind="ExternalInput").ap()
    wk_d = nc.dram_tensor("WK", [128, KCH * GD], BF, kind="ExternalInput").ap()
    wv_d = nc.dram_tensor("WV", [128, KCH * GD], BF, kind="ExternalInput").ap()
    # packed Wo.T slice: [128, 2*D], chunk kc at cols [kc*D, (kc+1)*D)
    wo_d = nc.dram_tensor("WO", [128, 2 * D], BF, kind="ExternalInput").ap()
    msk_d = None
    if n_mask:
        msk_d = nc.dram_tensor(
            "MSK", [n_mask, SKC, MCH], BF, kind="ExternalInput"
        ).ap()
    if with_bias:
        bq_d = nc.dram_tensor("BQ", [1, GD], BF, kind="ExternalInput").ap()
        bk_d = nc.dram_tensor("BK", [1, GD], BF, kind="ExternalInput").ap()
        bv_d = nc.dram_tensor("BV", [1, GD], BF, kind="ExternalInput").ap()
    y_d = nc.dram_tensor("Y", [S, D], F16, kind="ExternalOutput").ap()

    with tile.TileContext(nc) as tc, ExitStack() as top:
        const = top.enter_context(tc.tile_pool(name="const", bufs=1))

        wq_sb = const.tile([128, KCH * GD], BF, name="wq_sb", tag="wq_sb")
        wk_sb = const.tile([128, KCH * GD], BF, name="wk_sb", tag="wk_sb")
        wv_sb = const.tile([128, KCH * GD], BF, name="wv_sb", tag="wv_sb")
        wo_sb = const.tile([128, 2 * D], BF, name="wo_sb", tag="wo_sb")
        # per-chunk packed x tiles: chunk kk at cols [kk*SQC, (kk+1)*SQC)
        xq_r = [const.tile([128, KCH * SQC], BF, name=f"xq{sc}", tag=f"xq{sc}")
                for sc in range(NI)]
        xk_r = [const.tile([128, KCH * SQC], BF, name=f"xk{sc}", tag=f"xk{sc}")
                for sc in range(NI)]
        xv_r = [const.tile([128, KCH * SQC], BF, name=f"xv{sc}", tag=f"xv{sc}")
                for sc in range(NI)]

        # DMA order: weights + chunk-0 inputs first so proj(0) starts early,
        # then the rest in pipeline order, balanced across two rings
        def ld(eng, sb, dr):
            eng.dma_start(out=sb[:], in_=dr)

        # Three DMA rings (sync / gpsimd / scalar).  Transfers start as soon
        # as the rings are armed, in ring order, sharing HBM — so ring ORDER
        # is the only priority control.  Spread each chunk's three tensors
        # across the rings wave-by-wave: chunk c is complete by roughly
        # (c+1) * 9us and the pipeline is never input-starved.
        rsc = lambda sc: slice(sc * 128, (sc + 1) * 128)
        # Input stream on the sync+gpsimd rings ONLY.  A ring arms its next
        # transfer only when the previous completes, so the issuing ENGINE
        # is blocked in dma_start waits for the whole input stream — the
        # scalar (ACT) engine must never carry input transfers, or every
        # exp queues behind the armings (measured +20us on the exp start).
        # Global arrival order: w+qk0, qk1, v0, v1, qk2, v2, qk3, v3 — each
        # chunk's q/k (which gate the serial exp chain) land before its v
        # (needed only by the later AVs).
        half = KCH // 2 * SQC
        ghalf = KCH // 2 * GD
        for kks in range(KCH // 2):
            gd_c = slice(kks * 2 * GD, (kks + 1) * 2 * GD)
            sq_c = slice(kks * 2 * SQC, (kks + 1) * 2 * SQC)
            nc.sync.dma_start(out=wq_sb[:, gd_c], in_=wq_d[:, gd_c])
            nc.sync.dma_start(out=xq_r[0][:, sq_c], in_=xqT[rsc(0), sq_c])
            nc.gpsimd.dma_start(out=wk_sb[:, gd_c], in_=wk_d[:, gd_c])
            nc.gpsimd.dma_start(out=xk_r[0][:, sq_c], in_=xkT[rsc(0), sq_c])
        msk_sb = []
        for t in range(n_mask):
            m = const.tile([SKC, MCH], BF, name=f"msk{t}", tag=f"msk{t}")
            nc.gpsimd.dma_start(out=m[:], in_=msk_d[t])
            msk_sb.append(m)
        # v0 split across both rings right after q0/k0
        for hv, eng in ((0, nc.sync), (1, nc.gpsimd)):
            gc = slice(hv * ghalf, (hv + 1) * ghalf)
            hc = slice(hv * half, (hv + 1) * half)
            eng.dma_start(out=wv_sb[:, gc], in_=wv_d[:, gc])
            eng.dma_start(out=xv_r[0][:, hc], in_=xvT[rsc(0), hc])
        # chunk streams: q on sync / k on gpsimd so each chunk's q+k (which
        # gate the serial exp chain) land in parallel; v follows its chunk
        for hv in range(2):
            hc = slice(hv * half, (hv + 1) * half)
            nc.sync.dma_start(out=xq_r[1][:, hc], in_=xqT[rsc(1), hc])
            nc.gpsimd.dma_start(out=xk_r[1][:, hc], in_=xkT[rsc(1), hc])
        for hv in range(2):
            hc = slice(hv * half, (hv + 1) * half)
            nc.sync.dma_start(out=xv_r[1][:, hc], in_=xvT[rsc(1), hc])
            nc.gpsimd.dma_start(out=xk_r[2][:, hc], in_=xkT[rsc(2), hc])
        for hv in range(2):
            hc = slice(hv * half, (hv + 1) * half)
            nc.sync.dma_start(out=xq_r[2][:, hc], in_=xqT[rsc(2), hc])
            nc.gpsimd.dma_start(out=xk_r[3][:, hc], in_=xkT[rsc(3), hc])
        for hv in range(2):
            hc = slice(hv * half, (hv + 1) * half)
            nc.sync.dma_start(out=xq_r[3][:, hc], in_=xqT[rsc(3), hc])
        # wo/xv2/xv3 move to the scalar ring, one transfer armed per chunk
        # boundary from the main loop: each arming is instant (the previous
        # scalar transfer is long done), so the ACT engine loses ~0.6us per
        # boundary while sync/gpsimd shed 2.5MB of serial load

        if with_bias:
            onesrow = const.tile([1, SQC], BF, name="onesrow", tag="onesrow")
            nc.vector.memset(onesrow[:], 1.0)
            bq_sb = const.tile([1, GD], BF, name="bq_sb", tag="bq_sb")
            bk_sb = const.tile([1, GD], BF, name="bk_sb", tag="bk_sb")
            bv_sb = const.tile([1, GD], BF, name="bv_sb", tag="bv_sb")
            nc.sync.dma_start(out=bq_sb[:], in_=bq_d[:])
            nc.sync.dma_start(out=bk_sb[:], in_=bk_d[:])
            nc.sync.dma_start(out=bv_sb[:], in_=bv_d[:])

        # persistent activations
        acts = top.enter_context(tc.tile_pool(name="acts", bufs=1))
        qhT = [acts.tile([128, S], BF, name=f"qhT{p}", tag=f"qhT{p}")
               for p in range(NPAIR)]
        khT = [acts.tile([128, S], BF, name=f"khT{p}", tag=f"khT{p}")
               for p in range(NPAIR)]
        # v in natural layout, 65 cols per head (64 dims + ones column)
        vh = [acts.tile([128, GH * 65], BF, name=f"vh{j}", tag=f"vh{j}")
              for j in range(NJ)]
        # per-(chunk, head) 1/den rows at partition 64, persistent: normwo
        # reads them two chunks after they are written, so they must not
        # come from a recycling pool
        rcbS = acts.tile([65, 2 * GH * SQC], BF, name="rcbS", tag="rcbS")

        def rcb_ap(i, p, h):
            # two-chunk rotation: chunk i's rows are read by normwo(i) two
            # chunks later, just before chunk i+2 rewrites the same slots
            # (the framework's WAR dep on the earlier-emitted reads makes
            # the rotation safe)
            c0 = ((i % 2) * GH + 2 * p + h) * SQC
            return rcbS[64:65, c0 : c0 + SQC]
        for j in range(NJ):
            v3 = vh[j].rearrange("p (h x) -> p h x", h=GH)
            nc.vector.memset(v3[:, :, 64:65], 1.0)

        # ones on all 128 partitions; single rows are the lhsT of the K=1
        # denominator-broadcast matmuls (lhsT base must match rhs row base)
        onesP = const.tile([128, 128], BF, name="onesP", tag="onesP")
        nc.vector.memset(onesP[:], 1.0)
        # trigger the ~2.7us exp ACT_TABLE_LOAD now, under the input DMA
        # wait, instead of at the first real exp on the critical path
        wrm = const.tile([1, 2], BF, name="wrm", tag="wrm")
        nc.scalar.activation(wrm[:], onesP[0:1, 0:2], AF.Exp, scale=1.0)
        # moving operand for the PE warm-up dummies
        onesF2 = const.tile([128, SQC], BF, name="onesF2", tag="onesF2")
        nc.vector.memset(onesF2[:], 1.0)
        # f32 ones row for the denominator-gather matmuls (run as float32r)
        onesF = const.tile([65, 128], F32, name="onesF", tag="onesF")
        nc.vector.memset(onesF[:], 1.0)

        with (
            tc.tile_pool(name="pa", bufs=2, space="PSUM") as pa,
            tc.tile_pool(name="psc", bufs=2, space="PSUM") as psc,
            tc.tile_pool(name="pso", bufs=1, space="PSUM") as pso,
            tc.tile_pool(name="ex", bufs=5) as expool,
            tc.tile_pool(name="nrm", bufs=2) as nrm,
            tc.tile_pool(name="aou", bufs=10) as aoupool,
            tc.tile_pool(name="ao", bufs=3) as aopool,
            tc.tile_pool(name="yout", bufs=8) as ypool,
        ):
            def proj_units(sc):
                """Generator of PE filler units: the projections for chunk
                sc.  Unit order q0,k0,q1,k1,v0-3: qhT[m]/khT[m] belong to
                PAIR m, so after q0 is emitted the next chunk's pair-0
                scores are legal; k0 is needed only at j=4*sc, q1/k1 only
                when pair 1 starts, vh[4*sc+m] only at the matching AV —
                so all units after q0 can spill into the next chunk's
                attention as leftover filler."""
                cc = slice(sc * SQC, (sc + 1) * SQC)

                def qk_unit(m, which):
                    wt, xt, dstT, bias = (
                        (wq_sb, xq_r[sc], qhT, "BQ") if which == "q"
                        else (wk_sb, xk_r[sc], khT, "BK")
                    )
                    ps = pa.tile([128, SQC], F32, name="ps", tag="blk")
                    for kk in range(KCH):
                        wcol = slice(kk * GD + m * 128,
                                     kk * GD + (m + 1) * 128)
                        nc.tensor.matmul(
                            ps[:], wt[:, wcol],
                            xt[:, kk * SQC : (kk + 1) * SQC],
                            start=(kk == 0),
                            stop=(kk == KCH - 1) and not with_bias,
                        )
                        if kk == KCH // 2 - 1:
                            # half-unit yield: keeps the per-pull PE cost
                            # (~0.9us) below the exp instruction length so
                            # filler never starves the exp stream
                            yield
                    if with_bias:
                        b_sb = bq_sb if bias == "BQ" else bk_sb
                        nc.tensor.matmul(
                            ps[:], b_sb[:, m * 128 : (m + 1) * 128],
                            onesrow[:], start=False, stop=True,
                        )
                    # DVE, not ACT: the ACT engine is the exp stream's
                    # bottleneck and must not lose cycles to evacuations
                    nc.vector.tensor_copy(dstT[m][:, cc], ps[:])
                    yield

                def v_unit(m):
                    ps = pa.tile([128, SQC], F32, name="ps", tag="blk")
                    for kk in range(KCH):
                        nc.tensor.matmul(
                            ps[:, 0:GD],
                            xv_r[sc][:, kk * SQC + m * 128
                                     : kk * SQC + (m + 1) * 128],
                            wv_sb[:, kk * GD : (kk + 1) * GD],
                            start=(kk == 0),
                            stop=(kk == KCH - 1) and not with_bias,
                        )
                    if with_bias:
                        nc.tensor.matmul(
                            ps[:, 0:GD], onesrow[:, 0:128], bv_sb[:],
                            start=False, stop=True,
                        )
                    dst = vh[sc * 4 + m].rearrange("p (h x) -> p h x", h=GH)
                    src = ps[:, 0:GD].rearrange("p (h x) -> p h x", h=GH)
                    nc.vector.tensor_copy(dst[:, :, 0:64], src[:])
                    yield

                # under the lag-4 AV flush, v units are needed only at the
                # pair-0 flush (after jn3's fills), so q1/k1 go first: the
                # pair-boundary hoist and pair 1 read qhT[1]/khT[1] early,
                # and chunk 0's first three exps fire before the xv0 stall
                units = [qk_unit(0, "q"), qk_unit(0, "k"),
                         qk_unit(1, "q"), qk_unit(1, "k"),
                         v_unit(0), v_unit(1), v_unit(2), v_unit(3)]
                for u in units:
                    yield from u

            def emit_norm_pair(p, aoT, aoUs, rcb2, direct=False):
                """Broadcast 1/den + normalize for one head pair.  With
                `direct` the norm muls read psB straight from PSUM (skips
                the bcd staging copy — used on the kernel tail where the
                pa banks have no competing consumers)."""
                bcd = None
                if not direct:
                    bcd = nrm.tile([64, 2 * SQC], F32, name="bcd", tag="bcd")
                psBs = []
                for h in range(2):
                    rb = rcb2[h]  # [1, SQC] AP at partition 64
                    psB = pa.tile([64, SQC], F32, name="psB", tag="blk")
                    nc.tensor.matmul(
                        psB[:],
                        onesP[64:65, 0:64],
                        rb,
                        start=True, stop=True,
                        tile_position=(64, 0),
                    )
                    psBs.append(psB)
                if direct:
                    for h in range(2):
                        nc.vector.tensor_mul(
                            aoT[p][h * 64 : (h + 1) * 64, :],
                            aoUs[2 * p + h][0:64, :],
                            psBs[h][:],
                        )
                    return
                for h in range(2):
                    nc.vector.tensor_copy(
                        bcd[:, h * SQC : (h + 1) * SQC], psBs[h][:]
                    )
                for h in range(2):
                    nc.vector.tensor_mul(
                        aoT[p][h * 64 : (h + 1) * 64, :],
                        aoUs[2 * p + h][0:64, :],
                        bcd[:, h * SQC : (h + 1) * SQC],
                    )

            def normwo_units(state):
                """Generator of PE filler units: norm + Wo for a finished
                chunk (run late so the PE never waits on the reciprocal
                chain)."""
                i, aoT, aoUs, rcb_pairs = state
                for p, rcb2 in rcb_pairs.items():
                    emit_norm_pair(p, aoT, aoUs, rcb2, direct=(i == NI - 1))
                    yield
                for m in range(4):
                    rw = slice(m * 128, (m + 1) * 128)
                    orows = slice(i * SQC + m * 128, i * SQC + (m + 1) * 128)
                    for n in range(2):
                        ncol = slice(n * SQC, (n + 1) * SQC)
                        pY = pa.tile([128, SQC], F32, name="pY", tag="blk")
                        for kc in range(NPAIR):
                            nc.tensor.matmul(
                                pY[:],
                                aoT[kc][:, rw],
                                wo_sb[:, kc * D + n * SQC : kc * D + (n + 1) * SQC],
                                start=(kc == 0),
                                stop=(kc == NPAIR - 1),
                            )
                        y_sb = ypool.tile([128, SQC], F16, name="y_sb",
                                          tag="y_sb")
                        if i == NI - 1 and (m + n) % 2 == 0:
                            # ACT is idle after the last exp: alternate the
                            # tail's y evacuations between ACT and DVE
                            nc.scalar.copy(y_sb[:], pY[:])
                        else:
                            nc.vector.tensor_copy(y_sb[:], pY[:])
                        nc.sync.dma_start(out=y_d[orows, ncol], in_=y_sb[:])
                        yield

            def emit_sc_ij(i, p, j):
                """Scores + exp + mask for (chunk i, pair p, key block j).
                Returns (lo, hi, e) for the later AV."""
                c = cls[i][j]
                lo, hi = c["lo"], c["hi"]
                jw = slice(j * SKC, (j + 1) * SKC)
                iw = slice(i * SQC + lo, i * SQC + hi)
                # h0 in cols [0:SQC], h1 in cols [SQC:2*SQC]
                ps = psc.tile([128, 2 * SQC], F32, name="ps", tag="ps")
                e = expool.tile([128, 2 * SQC], BF, name="e", tag="e")
                for h in range(2):
                    pr = slice(h * 64, (h + 1) * 64)
                    nc.tensor.matmul(
                        ps[:, h * SQC + lo : h * SQC + hi],
                        khT[p][pr, jw],
                        qhT[p][pr, iw],
                        start=True, stop=True,
                    )
                ps3 = ps.rearrange("p (h c) -> p h c", h=2)
                e3 = e.rearrange("p (h c) -> p h c", h=2)
                nc.scalar.activation(
                    e3[:, :, lo:hi], ps3[:, :, lo:hi], AF.Exp,
                    scale=1.0 / np.sqrt(DK),
                )
                for c0, tidx in c["muls"]:
                    for h in range(2):
                        cw = slice(h * SQC + c0, h * SQC + c0 + MCH)
                        nc.vector.tensor_mul(
                            e[:, cw], e[:, cw], msk_sb[tidx][:]
                        )
                return lo, hi, e

            def first_j(i):
                return [j for j in range(NJ) if cls[i][j] is not None][0]

            def emit_attention(i, fill, pre0=None):
                """scores/exp/mask/AV + psO evacuation + reciprocal chain,
                with `fill()` pulling one PE filler unit per j step."""
                js = [j for j in range(NJ) if cls[i][j] is not None]
                assert js, "fully-masked query chunk not supported"
                aoT = [
                    aopool.tile([128, SQC], BF, name=f"aoT{p}", tag=f"aoT{p}")
                    for p in range(NPAIR)
                ]
                aoUs = []


                emit_sc = lambda p, j: emit_sc_ij(i, p, j)
                pre_sc = pre0  # first scores, hoisted by the caller
                for p in range(NPAIR):
                    psO = [
                        pso.tile([65, SQC], F32, name=f"psO{h}", tag=f"psO{h}")
                        for h in range(2)
                    ]

                    def emit_av(av):
                        jn, j, lo, hi, e = av
                        for h in range(2):
                            vcol = slice((2 * p + h) * 65, (2 * p + h + 1) * 65)
                            nc.tensor.matmul(
                                psO[h][:, lo:hi],
                                vh[j][:, vcol],
                                e[:, h * SQC + lo : h * SQC + hi],
                                start=(jn == 0), stop=(jn == len(js) - 1),
                            )

                    # AV matmuls are emitted two j behind the scores matmuls:
                    # the in-order PE can then run scores_{j+1} while the ACT
                    # engine computes exp_j, and an AV's semaphores are long
                    # since set when the PE reaches it (no pipeline drain).
                    pend_av = []
                    for jn, j in enumerate(js):
                        if i == NI - 1 and p == 1 and jn == min(6, len(js) - 1):
                            # pair 0's normalize, late enough that its
                            # reciprocal chain (started at the pair
                            # boundary) is long since done: only pair 1's
                            # chain + Wo remain after the last AV
                            emit_norm_pair(0, aoT, aoUs,
                                           [rcb_ap(i, 0, 0), rcb_ap(i, 0, 1)])
                        if jn == 0 and pre_sc is not None:
                            lo, hi, e = pre_sc
                            pre_sc = None
                        else:
                            lo, hi, e = emit_sc(p, j)
                        if len(pend_av) >= (4 if i <= 1 else 2):
                            # deeper AV lag on the first chunks: their v
                            # tensors arrive after q/k, and the deferred AVs
                            # let the exp stream run ahead of the v DMA
                            emit_av(pend_av.pop(0))
                        fill()
                        if i == 0:
                            # chunk 0 has only 8 j-steps; double-pull so all
                            # of proj(1) lands inside it instead of draining
                            # at the boundary
                            fill()
                        pend_av.append((jn, j, lo, hi, e))
                    # flush the lag-2 AVs; the next pair's first scores slot
                    # between them so its exp starts the moment this pair's
                    # last exp ends (the PE would otherwise idle ~1us here
                    # waiting on exp(last) before the final AV)
                    if len(pend_av) >= 2 and p + 1 < NPAIR:
                        emit_av(pend_av.pop(0))
                        pre_sc = emit_sc(p + 1, js[0])
                    for av in pend_av:
                        emit_av(av)
                    if i == NI - 1 and p == 1:
                        # keep the PE clock ramped through the tail's
                        # reciprocal/normalize chain so the final Wo
                        # matmuls run at full speed
                        ps_d = psc.tile([128, 2 * SQC], F32, name="ps",
                                        tag="ps")
                        for r in range(5):
                            rr = (r % 2) * SQC
                            nc.tensor.matmul(
                                ps_d[:, rr : rr + SQC],
                                onesP[:], onesF2[:],
                                start=True, stop=True,
                            )
                    # evacuate promptly (frees the psO banks); row 64 is the
                    # softmax denominator.  On the tail (last chunk, pair 1)
                    # ACT is idle — split the two copies across engines.
                    for h in range(2):
                        aoU = aoupool.tile([65, SQC], F32, name="aoU", tag="aoU")
                        if i == NI - 1 and p == 1 and h == 0:
                            nc.scalar.copy(aoU[:], psO[h][:])
                        else:
                            nc.vector.tensor_copy(aoU[:], psO[h][:])
                        aoUs.append(aoU)
                    # per-head reciprocal straight off each aoU's denominator
                    # row — no cross-partition DMA gather anywhere (the
                    # sync/gpsimd rings are busy arming the input stream for
                    # ~60us and the scalar engine must stay free for exp).
                    # Full-tile op at partition base 0 (a custom-DVE AP based
                    # at partition 64 miscompiles); input must be SBUF, not
                    # PSUM (the approx reciprocal is an fp32 bit trick and
                    # PSUM's raw accumulator format breaks it on HW).  Only
                    # row 64 (the denominator) is consumed, into the
                    # persistent rcbS store read by normwo two chunks later.
                    for h in range(2):
                        rc_r = nrm.tile([65, SQC], F32, name="rc_r",
                                        tag="rc_t")
                        nc.vector.reciprocal_approx_fast(
                            rc_r[:], aoUs[2 * p + h][:]
                        )
                        nc.vector.tensor_copy(
                            rcb_ap(i, p, h), rc_r[64:65, :]
                        )
                if i == NI - 1:
                    # pair 0's norm was emitted inline during pair 1
                    return i, aoT, aoUs, {1: [rcb_ap(i, 1, 0),
                                              rcb_ap(i, 1, 1)]}
                return i, aoT, aoUs, {0: [rcb_ap(i, 0, 0), rcb_ap(i, 0, 1)],
                                      1: [rcb_ap(i, 1, 0), rcb_ap(i, 1, 1)]}

            class Filler:
                """Chain of (generator, eligibility) PE work sources.  A
                generator yields nothing until the global pull counter
                exceeds its eligibility, so a unit whose input DMA has not
                landed yet cannot head-of-line-block the attention stream
                on the in-order PE."""
                def __init__(self, *gens):
                    self.gens = [[g, e, 0] for g, e in gens if g is not None]
                    self.k = 0

                def __call__(self):
                    self.k += 1
                    i = 0
                    while i < len(self.gens):
                        g, e, _ = self.gens[i]
                        if self.k <= e:
                            i += 1
                            continue
                        try:
                            next(g)
                            self.gens[i][2] += 1
                            return True
                        except StopIteration:
                            self.gens.pop(i)
                    return False

                def pull_upto(self, gen, n):
                    """Force-emit units of `gen` until n total have been
                    pulled from it (ignores eligibility)."""
                    for ge in list(self.gens):
                        if ge[0] is gen:
                            try:
                                while ge[2] < n:
                                    next(gen)
                                    ge[2] += 1
                            except StopIteration:
                                self.gens.remove(ge)

                def leftovers(self):
                    out = [(g, 0) for g, _, _ in self.gens]
                    self.gens = []
                    return out

                def drain(self):
                    for ge in self.gens:
                        ge[1] = 0
                    while self():
                        pass

            # proj(0) up front (DMA-gated), then chunk i's attention with
            # norm+Wo (two chunks late, so the DMA-gathered reciprocal is
            # always long since done) and proj(i+1) as PE filler.  At each
            # chunk boundary only proj(i+1)'s q/k half-units are forced
            # (attn(i+1)'s scores read qhT/khT); the v units spill over as
            # leftover filler pulled during attn(i+1)'s first j-steps
            # (vh[4(i+1)+m] is first read by an AV emitted at jn>=6), so
            # neither the PE nor the exp stream drains at chunk boundaries.
            #
            # Dummy matmuls interleave with the DMA-gated proj(0) units: the
            # PE clock ramps 0.65->2.4GHz only across ~8us of CONTINUOUS
            # execution, so keeping it busy while input slices trickle in
            # makes the post-DMA sprint run at full clock instead of
            # restarting the ramp at every slice boundary.  They write a
            # psc-pool scratch tile (psc is untouched until attention;
            # pa holds an open proj accumulation and must not be cycled).
            def warm_units(n_groups, per_group):
                for _ in range(n_groups):
                    ps_d = psc.tile([128, 2 * SQC], F32, name="ps", tag="ps")
                    for r in range(per_group):
                        rr = (r % 2) * SQC
                        nc.tensor.matmul(
                            ps_d[:, rr : rr + SQC],
                            onesP[:], onesF2[:],
                            start=True, stop=True,
                        )
                    yield

            # Interleave warm groups with chunk 0's q0/k0 only (4 half-unit
            # pulls); the remaining chunk-0 units spill into attn(0), so the
            # first exp fires as soon as q0+k0's DMA lands instead of after
            # the whole wave-0.  Leftover warm groups are dropped, not
            # drained — they must not delay the hoisted first scores.
            p0 = proj_units(0)
            wf = Filler((warm_units(12, 4), 0))
            for _ in range(6):
                wf()
            for _ in range(4):
                next(p0)
                wf()
            pend = []
            carry = [(p0, 0)]
            pre = emit_sc_ij(0, 0, first_j(0))
            for i in range(NI):
                # prev chunk's leftovers come first; v tensors land mid-chunk
                # (xv1 on sync, xv2/xv3 via the late scalar armings), so the
                # v-unit pulls are delayed past their arrival — with the AV
                # lag they are only needed at the flush / the diagonal AVs
                gens = [(g, {1: 4, 2: 6, 3: 8}.get(i, e)) for g, e in carry]
                if i >= 2:
                    gens.append((normwo_units(pend.pop(0)), 0))
                    if i == NI - 1:
                        gens.append((normwo_units(pend.pop(0)), 0))
                nxt = proj_units(i + 1) if i + 1 < NI else None
                if nxt is not None:
                    # eligibility tuned to x(i+1)'s DMA arrival so a pulled
                    # unit rarely head-of-line-blocks the PE: chunk 2's x
                    # lands ~2/3 through attn(1), later chunks are covered
                    # by the normwo units queued ahead of them
                    # i==2: only proj(3)'s q/k half-units fit under exp(2);
                    # its v units carry into attn(3), which has ~10us of
                    # exp slack to absorb them
                    gens.append((nxt, 14 if i == 0 else 12 if i == 1 else 16))
                fill = Filler(*gens)
                state = emit_attention(i, fill, pre)
                pend.append(state)
                # one late-needed transfer per chunk boundary on the scalar
                # ring: the arming is instant (the previous scalar transfer
                # is long done) so the ACT engine loses only ~0.6us here,
                # while sync/gpsimd shed 2.5MB of serialized input load
                if i == 0:
                    nc.scalar.dma_start(out=wo_sb[:], in_=wo_d[:])
                elif i == 1:
                    nc.scalar.dma_start(out=xv_r[2][:], in_=xvT[rsc(2), :])
                elif i == 2:
                    nc.scalar.dma_start(out=xv_r[3][:], in_=xvT[rsc(3), :])
                if nxt is not None:
                    fill.pull_upto(nxt, 8)  # q0,k0,q1,k1 half-units
                    # chunk-boundary hoist: with qhT[0]/khT[0] of chunk i+1
                    # now emitted, the next chunk's first scores+exp go out
                    # before this chunk's trailing norm work so the exp
                    # stream continues across the boundary
                    pre = emit_sc_ij(i + 1, 0, first_j(i + 1))
                else:
                    pre = None
                carry = fill.leftovers()
            Filler(*carry).drain()
            Filler((normwo_units(pend.pop(0)), 0)).drain()

    nc.compile()
    return nc


def _cls_sig(cls):
    out = []
    for row in cls:
        for c in row:
            if c is None:
                out.append(None)
            else:
                out.append((c["lo"], c["hi"], tuple(c["muls"])))
    return tuple(out)


def kernel(q, k, v, Wq, bq, Wk, bk, Wv, bv, Wo, bo, mask):
    global LAST_EXEC_NS, LAST_RESULT
    from concourse.bass_utils import run_bass_kernel_spmd

    q = np.asarray(q, np.float32)
    k = np.asarray(k, np.float32)
    v = np.asarray(v, np.float32)
    mask_st = np.asarray(mask).reshape(S, S).astype(bool)

    cls, mtiles = _classify_mask(mask_st)
    with_bias = not (
        np.all(np.asarray(bq) == 0)
        and np.all(np.asarray(bk) == 0)
        and np.all(np.asarray(bv) == 0)
    )

    sig = (_cls_sig(cls), len(mtiles), with_bias)
    if sig not in _prog_cache:
        _prog_cache[sig] = _build(cls, len(mtiles), with_bias)
    nc = _prog_cache[sig]

    def pack_w(wt, gd):  # [nch*128, gd] -> [128, nch*gd]
        nch = wt.shape[0] // 128
        return np.ascontiguousarray(
            wt.reshape(nch, 128, gd).transpose(1, 0, 2).reshape(128, nch * gd)
        ).astype(_BF)

    def pack_x(xb):  # [S, D] -> [NI*128, KCH*SQC], block (sc, kk)
        xt = xb.T.reshape(KCH, 128, NI, SQC)       # [kk, row, sc, col]
        return np.ascontiguousarray(
            xt.transpose(2, 1, 0, 3).reshape(NI * 128, KCH * SQC)
        ).astype(_BF)

    in_maps = []
    for c in range(NCORE):
        b, g = divmod(c, TPG)
        rows = slice(g * GD, (g + 1) * GD)
        im = {
            "xqT": pack_x(q[b]),
            "xkT": pack_x(k[b]),
            "xvT": pack_x(v[b]),
            "WQ": pack_w(np.ascontiguousarray(Wq[rows, :].T), GD),
            "WK": pack_w(np.ascontiguousarray(Wk[rows, :].T), GD),
            "WV": pack_w(np.ascontiguousarray(Wv[rows, :].T), GD),
            "WO": pack_w(np.ascontiguousarray(Wo[:, rows].T), D),
        }
        if mtiles:
            im["MSK"] = np.stack(mtiles)
        if with_bias:
            im["BQ"] = np.asarray(bq)[rows].reshape(1, GD).astype(_BF)
            im["BK"] = np.asarray(bk)[rows].reshape(1, GD).astype(_BF)
            im["BV"] = np.asarray(bv)[rows].reshape(1, GD).astype(_BF)
        in_maps.append(im)

    res = run_bass_kernel_spmd(nc, in_maps, list(range(NCORE)), trace=TRACE)
    LAST_RESULT = res
    LAST_EXEC_NS = res.exec_time_ns

    out = np.zeros((B, S, D), np.float32)
    for c in range(NCORE):
        out[c // TPG] += res.results[c]["Y"].astype(np.float32)
    out += np.asarray(bo, np.float32)
    return out

